# revision 24
# baseline (speedup 1.0000x reference)
"""Encoder-decoder LSTM seq2seq loss kernel for 8 TRN2 NeuronCores.

Strategy (v2):
  - x-part gate contributions (x @ W_ih^T + b_ih + b_hh) are precomputed on
    the HOST for all 95 steps and streamed to SBUF in 8-step windows; the
    device recurrence only does the h-part matmuls (64 [128,128]x[128,64]
    chunk matmuls per step, moving-bound at ~29ns each) plus a 3-instruction
    identity-matmul injection of the x-part into the gate PSUM tiles.
  - Tanh-only gates: sigmoid(x) = 0.5*tanh(x/2)+0.5 is folded into the
    fused DVE op affine_mul_reduce ((in0*s+b)*in1), so the whole kernel
    uses only Tanh/Exp/Copy activations -> a single ACT table
    (exp_and_others), letting decoder-phase logits Exp interleave with
    recurrence Tanh with zero table reloads.
  - The 47-step decoder logits (vocab sharded 8 ways, 4096 rows padded)
    run as [128sb x 1024v] PSUM tiles: a K=1 "bias matmul" (ones x b_out
    row) initializes the accumulation group, 4 k-chunk matmuls accumulate
    h.W, then ACT Exp in-place with the free-axis accumulator produces the
    per-(sb-chunk, vtile) sum-exp directly into out_s. Tiles are pumped
    into the decoder step stream as soon as their 2 source steps are done,
    filling the PE idle gaps left by the ACT/DVE recurrence tail.
  - Target logits are computed on the HOST from the DMA'd-out decoder
    hidden states (3008x512 dot, trivial).
"""

import sys

sys.path.insert(0, "/opt/trn_rl_repo")

import numpy as np
import ml_dtypes

BF16 = ml_dtypes.bfloat16

# Model dims (hardcoded per contract)
SRC, TGT, B, H, V = 48, 48, 64, 512, 32000
DEC = TGT - 1                  # 47 decoder steps
SB = DEC * B                   # 3008 (step*batch)
SBC = 24                       # ceil(3008/128) sb-chunks
SBP = SBC * 128                # 3072 padded
NCORES = 8
VSH = V // NCORES              # 4000 vocab rows per core
VSP = 4096                     # padded shard
WIN = 4                        # gx window (steps)
NG = 16                        # gate chunks (2048/128)
KC = 4                         # hidden chunks (512/128)
TSTEPS = SRC + DEC             # 95
NWIN = 24                      # ceil(95/4)
WCOLS = WIN * B                # 256 cols per window
VT = 1024                      # logits vtile width
NVT = VSP // VT                # 4 vtiles per sb chunk

# gate-chunk indices in the permuted [i f o g] weight layout
I_CH = list(range(0, 4))
F_CH = list(range(4, 8))
O_CH = list(range(8, 12))
G_CH = list(range(12, 16))

_COMPILED = None


def _build():
    import concourse.bass as bass
    import concourse.bacc as bacc
    import concourse.tile as tile
    from concourse import mybir

    f32 = mybir.dt.float32
    bf16 = mybir.dt.bfloat16
    AF = mybir.ActivationFunctionType

    nc = bacc.Bacc("TRN2", target_bir_lowering=False, debug=False,
                   num_devices=NCORES)

    def din(name, shape, dt=bf16):
        return nc.dram_tensor(name, shape, dt, kind="ExternalInput").ap()

    fp8 = mybir.dt.float8e4

    gxw_in = din("gxw", [NWIN, 128, NG * WCOLS])
    wh_e = din("wh_e", [KC, 128, 4 * H])
    wh_d = din("wh_d", [KC, 128, 4 * H])
    mask_in = din("mask", [128, SRC * KC * B], mybir.dt.uint8)
    ident_in = din("ident", [128, 128])
    ones1_in = din("ones1", [1, 128])
    wot_in = din("wot", [KC // 2, 128, 2 * VSP], fp8)
    boutr_in = din("boutr", [1, VSP])

    ht_out = nc.dram_tensor("ht_out", [128, KC * SB], bf16,
                            kind="ExternalOutput").ap()
    out_s = nc.dram_tensor("out_s", [128, SBC * NVT], f32,
                           kind="ExternalOutput").ap()

    with tile.TileContext(nc) as tc:
        from contextlib import ExitStack
        with ExitStack() as ctx:
            # ---- pools ----
            pconst = ctx.enter_context(tc.tile_pool(name="const", bufs=1))
            pht = ctx.enter_context(tc.tile_pool(name="ht", bufs=1))
            pw = ctx.enter_context(tc.tile_pool(name="w", bufs=1))
            pgx = ctx.enter_context(tc.tile_pool(name="gx", bufs=2))
            pstate = ctx.enter_context(tc.tile_pool(name="state", bufs=3))
            pact = ctx.enter_context(tc.tile_pool(name="act", bufs=2))

            # ---- constants / static loads (ordered for fast step-0) ----
            ident = pconst.tile([128, 128], bf16)
            nc.sync.dma_start(ident[:], ident_in[:])
            ones1 = pconst.tile([1, 128], bf16)
            nc.sync.dma_start(ones1[:], ones1_in[:])
            s_all = pconst.tile([128, SBC * NVT], f32)
            dummy = pconst.tile([128, 1], f32)

            # gx window 0 + encoder weights first: step 0 needs them
            gx_cur = pgx.tile([128, NG * WCOLS], bf16, tag="gx")
            nc.sync.dma_start(gx_cur[:], gxw_in[0])

            def load_w(dram, tag, width, n=KC, dt=bf16):
                ts = []
                for k in range(n):
                    t = pw.tile([128, width], dt, tag=f"{tag}{k}")
                    nc.sync.dma_start(t[:], dram[k])
                    ts.append(t)
                return ts

            we_h = load_w(wh_e, "whe", 4 * H)
            wd_h = [None] * KC  # loaded mid-encoder, off the startup path

            # masks split in two tiles so step 0's restore isn't gated on
            # the whole 1.5MB transfer
            MSPLIT = 16 * KC * B
            mask_a = pconst.tile([128, MSPLIT], mybir.dt.uint8)
            nc.sync.dma_start(mask_a[:], mask_in[:, 0:MSPLIT])
            mask_b = pconst.tile([128, SRC * KC * B - MSPLIT],
                                 mybir.dt.uint8)
            nc.sync.dma_start(mask_b[:], mask_in[:, MSPLIT:])

            def mask_sl(t):
                if t < 16:
                    return mask_a[:, t * KC * B:(t + 1) * KC * B]
                return mask_b[:, (t - 16) * KC * B:(t - 15) * KC * B]

            gx_nxt = pgx.tile([128, NG * WCOLS], bf16, tag="gx")
            nc.sync.dma_start(gx_nxt[:], gxw_in[1])

            # decoder hidden states, transposed: col = k*SBP + t*64 + b
            # bf16 copy feeds the recurrence + host target-logits;
            # fp8 copy feeds the DoubleRow logits GEMM
            ht = pht.tile([128, KC * SBP], bf16)
            ht8 = pht.tile([128, KC * SBP], fp8)
            for k in range(KC):
                nc.vector.memset(ht[:, k * SBP + SB:(k + 1) * SBP], 0.0)
                nc.vector.memset(ht8[:, k * SBP + SB:(k + 1) * SBP], 0.0)

            h_prev = pstate.tile([128, KC * B], bf16, tag="h")
            nc.vector.memset(h_prev[:], 0.0)
            c_prev = pstate.tile([128, 256], f32, tag="c")
            nc.vector.memset(c_prev[:], 0.0)

            wot_t = [None] * KC
            boutr = None

            with (
                tc.tile_pool(name="psA", bufs=1, space=bass.MemorySpace.PSUM)
                    as psA,
                tc.tile_pool(name="psB", bufs=1, space=bass.MemorySpace.PSUM)
                    as psB,
                tc.tile_pool(name="psC", bufs=1, space=bass.MemorySpace.PSUM)
                    as psC,
                tc.tile_pool(name="psL", bufs=2, space=bass.MemorySpace.PSUM)
                    as psL,
                tc.tile_pool(name="psF", bufs=1, space=bass.MemorySpace.PSUM)
                    as psF,
            ):
                AMR = nc.vector.affine_mul_reduce

                def pe_keepalive(src):
                    """Tiny matmul dependent on an ACT-tail output: fires
                    mid-tail so the PE never idles long enough to drop out
                    of its max P-state. All-f32 (src is an f32 ACT tile)."""
                    pf = psF.tile([128, 256], f32, tag="psF")
                    s = src[:, 0:128]
                    nc.tensor.matmul(pf[:, 0:128], s, s,
                                     start=True, stop=True)

                def lstm_step(gx, lt, h_rhs, c_prev, wh_t, h_out,
                              keepalive=False):
                    """One step. g chunks first (c-path overlaps i/f/o
                    matmuls), o last (short h tail). h_out(sgo, tnc)
                    writes the new h."""
                    pA = psA.tile([128, 512], f32, tag="psA")  # i|f
                    pB = psB.tile([128, 256], f32, tag="psB")  # g
                    pC = psC.tile([128, 256], f32, tag="psC")  # o

                    def dst(c):
                        if c in G_CH:
                            return pB[:, (c - 12) * B:(c - 11) * B]
                        if c in O_CH:
                            return pC[:, (c - 8) * B:(c - 7) * B]
                        return pA[:, c * B:(c + 1) * B]
                    order = G_CH + I_CH + F_CH + O_CH
                    # gx window layout is lt-major: [p, lt*1024 + g*64 + b]
                    gx_r = gx[:].rearrange("p (l g s) -> p l g s",
                                           l=WIN, g=NG)
                    # x-part injection (identity stationary, one wide matmul
                    # per PSUM tile; starts each accumulation group)
                    nc.tensor.matmul(
                        pB[:].rearrange("p (g s) -> p g s", g=4),
                        ident[:], gx_r[:, lt, 12:16, :],
                        start=True, stop=False)
                    nc.tensor.matmul(
                        pA[:].rearrange("p (g s) -> p g s", g=8),
                        ident[:], gx_r[:, lt, 0:8, :],
                        start=True, stop=False)
                    nc.tensor.matmul(
                        pC[:].rearrange("p (g s) -> p g s", g=4),
                        ident[:], gx_r[:, lt, 8:12, :],
                        start=True, stop=False)
                    # h-part; last matmul into each tile carries stop
                    for c in order:
                        for k in range(KC):
                            last = (k == KC - 1) and c in (15, 7, 11)
                            nc.tensor.matmul(
                                dst(c),
                                wh_t[k][:, c * 128:(c + 1) * 128],
                                h_rhs(k),
                                start=False, stop=last)
                    # ACT: tanh-only gates. th_x = tanh(x/2) for i,f,o;
                    # th_g = tanh(g). sigmoid(x) = 0.5*th_(x/2) + 0.5 is
                    # folded into the AMR scale/bias.
                    tng = pact.tile([128, 256], f32, tag="tng")
                    nc.scalar.activation(tng[:], pB[:], AF.Tanh)
                    tif = pact.tile([128, 512], f32, tag="tif")
                    nc.scalar.activation(tif[:], pA[:], AF.Tanh, scale=0.5)
                    tho = pact.tile([128, 256], f32, tag="tho")
                    nc.scalar.activation(tho[:], pC[:], AF.Tanh, scale=0.5)
                    if keepalive:
                        pe_keepalive(tif)
                    # c2 = sig(f)*c + sig(i)*tanh(g)
                    t2 = pact.tile([128, 256], f32, tag="t2")
                    AMR(t2[:], dummy[:], tif[:, 0:256], tng[:], 0.5, 0.5)
                    t1 = pact.tile([128, 256], f32, tag="t1")
                    AMR(t1[:], dummy[:], tif[:, 256:512], c_prev[:], 0.5, 0.5)
                    c_new = pstate.tile([128, 256], f32, tag="c")
                    nc.vector.tensor_add(c_new[:], t1[:], t2[:])
                    tnc = pact.tile([128, 256], f32, tag="tnc")
                    nc.scalar.activation(tnc[:], c_new[:], AF.Tanh)
                    if keepalive:
                        pe_keepalive(tho)
                        pe_keepalive(tnc)
                    h_out(tho, tnc)
                    return c_new

                # --------- logits piece machinery ---------
                plist = [(sb, v) for sb in range(SBC) for v in range(NVT)]
                pptr = [0]

                ht8_r = ht8[:].rearrange("p (k s) -> p k s", k=KC)

                def run_piece(sb, v):
                    # fp8 DoubleRow GEMM: each mm contracts 2 k-tiles.
                    # lhsT = ht8 [128, 2, 128sb], rhs = wot8 [128, 2, 512v]
                    pl = psL.tile([128, VT], f32, tag="psL")
                    for hf in range(VT // 512):  # matmul out <= 512 f32
                        c0 = v * VT + hf * 512
                        nc.tensor.matmul(pl[:, hf * 512:hf * 512 + 512],
                                         ones1[:], boutr[:, c0:c0 + 512],
                                         start=True, stop=False)
                        for kp in range(KC // 2):
                            nc.tensor.matmul(
                                pl[:, hf * 512:hf * 512 + 512],
                                ht8_r[:, 2 * kp:2 * kp + 2,
                                      sb * 128:(sb + 1) * 128],
                                wot_t[kp][:].rearrange(
                                    "p (i v) -> p i v", i=2)[:, :,
                                                            c0:c0 + 512],
                                start=False, stop=(kp == KC // 2 - 1),
                                perf_mode=mybir.MatmulPerfMode.DoubleRow,
                                skip_group_check=True)
                    col = sb * NVT + v
                    nc.scalar.activation(pl[:], pl[:], AF.Exp,
                                         accum_out=s_all[:, col:col + 1])

                def pump(ready_chunks, max_n):
                    n = 0
                    while pptr[0] < len(plist) and n < max_n:
                        sb, v = plist[pptr[0]]
                        if sb >= ready_chunks:
                            break
                        run_piece(sb, v)
                        pptr[0] += 1
                        n += 1

                # ============ unified 95-step recurrence ============
                for t in range(TSTEPS):
                    w, lt = t // WIN, t % WIN
                    if lt == 0 and w > 0:
                        gx_cur = gx_nxt
                        if w + 1 < NWIN:
                            gx_nxt = pgx.tile([128, NG * WCOLS], bf16,
                                              tag="gx")
                            nc.sync.dma_start(gx_nxt[:], gxw_in[w + 1])
                    if t == 20:
                        # decoder weights, off the startup critical path
                        wd_l = load_w(wh_d, "whd", 4 * H)
                        for k in range(KC):
                            wd_h[k] = wd_l[k]
                    if t == 40:
                        # prefetch vocab shard while encoder still runs
                        wot_l = load_w(wot_in, "wot", 2 * VSP, n=KC // 2,
                                       dt=fp8)
                        for kp in range(KC // 2):
                            wot_t[kp] = wot_l[kp]
                        boutr = pconst.tile([1, VSP], bf16)
                        nc.sync.dma_start(boutr[:], boutr_in[:])

                    enc = t < SRC
                    d = t - SRC  # decoder step index (valid if not enc)
                    if enc or d == 0:
                        hp = h_prev
                        rhs = (lambda k, hp=hp: hp[:, k * B:(k + 1) * B])
                    else:
                        rhs = (lambda k, tp=d - 1:
                               ht[:, k * SBP + tp * B:
                                  k * SBP + (tp + 1) * B])

                    if enc:
                        h_new = pstate.tile([128, KC * B], bf16, tag="h")

                        def h_out(tho, tnc, h_new=h_new):
                            AMR(h_new[:], dummy[:], tho[:], tnc[:], 0.5, 0.5)
                    else:
                        def h_out(tho, tnc, d=d):
                            # write k-chunks separately (contiguous outs),
                            # k0 first so next step's first matmuls unblock
                            for k in range(KC):
                                AMR(ht[:, k * SBP + d * B:
                                       k * SBP + (d + 1) * B],
                                    dummy[:],
                                    tho[:, k * B:(k + 1) * B],
                                    tnc[:, k * B:(k + 1) * B], 0.5, 0.5)
                            # fp8 shadow copy for the DoubleRow logits GEMM
                            nc.vector.tensor_copy(
                                ht8_r[:, :, d * B:(d + 1) * B],
                                ht[:].rearrange("p (k s) -> p k s",
                                                k=KC)[:, :,
                                                      d * B:(d + 1) * B])

                    wh_t = we_h if enc else wd_h
                    c_new = lstm_step(gx_cur, lt, rhs, c_prev, wh_t, h_out,
                                      keepalive=(t < SRC + 3))

                    if enc:
                        # h-restore only: the c-restore is handled by
                        # host-side gx pad poisoning (f->+30, i->-30 makes
                        # c2 = c1 naturally on pad steps)
                        nc.vector.copy_predicated(h_new[:], mask_sl(t),
                                                  h_prev[:])
                        h_prev = h_new
                    c_prev = c_new

                    if not enc and d >= 2:
                        pump((d + 1) // 2, 3)

                # ht out for host-side target logits (overlaps piece drain)
                for k in range(KC):
                    nc.sync.dma_start(ht_out[:, k * SB:(k + 1) * SB],
                                      ht[:, k * SBP:k * SBP + SB])
                # drain remaining logits pieces
                pump(SBC, len(plist))
                nc.sync.dma_start(out_s[:], s_all[:])

    nc.compile()
    return nc


def _prep(inputs):
    """Host-side data prep. Returns per-core in_maps + host combine data."""
    il = np.asarray(inputs["input_lines"])
    tl = np.asarray(inputs["target_lines"])
    f = lambda k: np.asarray(inputs[k], np.float32)
    emb_in, emb_tgt = f("emb_in").copy(), f("emb_tgt").copy()
    emb_in[0] = 0.0
    emb_tgt[0] = 0.0
    W_out, b_out = f("W_out"), f("b_out")

    perm = np.concatenate([np.arange(0, 512), np.arange(512, 1024),
                           np.arange(1536, 2048), np.arange(1024, 1536)])

    def wt(w):  # [2048,512] -> [4,128,2048] bf16 (transposed, gate-permuted)
        return np.ascontiguousarray(
            w[perm].T.reshape(KC, 128, 4 * H)).astype(BF16)

    # x-part gates for all steps, biases folded in, gate-permuted
    x_enc = emb_in[il.reshape(-1)]                       # [3072, 512]
    g_enc = x_enc @ f("W_ih_e").T + (f("b_ih_e") + f("b_hh_e"))
    g_enc = g_enc[:, perm]
    # pad poisoning: on pad steps force sig(f)=1, sig(i)=0 so c2 = c1
    # without a device-side c-restore (h still needs its predicated copy)
    pad = (il.reshape(-1) == 0)
    g_enc[pad, 0:512] = -30.0                            # i gates
    g_enc[pad, 512:1024] = 30.0                          # f gates
    tgt_in = tl[:DEC].reshape(-1)
    x_dec = emb_tgt[tgt_in]                              # [3008, 512]
    g_dec = x_dec @ f("W_ih_d").T + (f("b_ih_d") + f("b_hh_d"))
    g_all = np.zeros((NWIN * WCOLS, 4 * H), np.float32)  # [6144, 2048]
    g_all[:SRC * B] = g_enc
    g_all[SRC * B:SRC * B + SB] = g_dec[:, perm]
    # windowed lt-major layout: [win, 128, lt*1024 + gate_chunk*64 + b]
    gxw = np.ascontiguousarray(
        g_all.reshape(NWIN, WIN, B, NG, 128).transpose(0, 4, 1, 3, 2)
        .reshape(NWIN, 128, NG * WCOLS)).astype(BF16)

    m = (il == 0).astype(np.uint8)                       # [48, 64]
    mask = np.ascontiguousarray(
        np.broadcast_to(m[:, None, None, :], (SRC, 128, KC, B))
        .transpose(1, 0, 2, 3).reshape(128, SRC * KC * B)).astype(np.uint8)

    tgt_next = tl[1:TGT].reshape(-1)                     # [3008]
    w_tgt = W_out[tgt_next].astype(np.float64)           # [3008, 512]
    b_tgt = b_out[tgt_next].astype(np.float64)

    common = dict(
        gxw=gxw,
        wh_e=wt(f("W_hh_e")), wh_d=wt(f("W_hh_d")),
        mask=mask,
        ident=np.eye(128, dtype=BF16),
        ones1=np.ones((1, 128), BF16),
    )
    FP8 = ml_dtypes.float8_e4m3fn
    in_maps = []
    for c in range(NCORES):
        ws = np.zeros((VSP, H), np.float32)
        ws[:VSH] = W_out[c * VSH:(c + 1) * VSH]
        bs = np.full(VSP, -88.0, np.float32)
        bs[:VSH] = b_out[c * VSH:(c + 1) * VSH]
        # DoubleRow layout: wot8[kpair, p, i, v] = W^T[(2kp+i)*128+p, v]
        wsT = ws.T.reshape(2, 2, 128, VSP)               # [kp, i, p, v]
        wot8 = np.ascontiguousarray(
            wsT.transpose(0, 2, 1, 3).reshape(2, 128, 2 * VSP)).astype(FP8)
        in_maps.append(dict(
            common,
            wot=wot8,
            boutr=np.ascontiguousarray(bs.reshape(1, VSP)).astype(BF16),
        ))
    return in_maps, w_tgt, b_tgt


def _combine(results, w_tgt, b_tgt):
    # sum-exp partials: out_s[p, sb*4+v]; sb index = chunk*128 + p
    s = np.zeros((128, SBC), np.float64)
    for r in results:
        os = np.asarray(r["out_s"], np.float64).reshape(128, SBC, NVT)
        s += os.sum(axis=2)
    s = s.T.reshape(-1)[:SB]                             # [3008]
    lse = np.log(s)
    # target logits from device hidden states
    hto = np.asarray(results[0]["ht_out"], np.float64)   # [128, 4*3008]
    Hm = hto.reshape(128, KC, SB).transpose(2, 1, 0).reshape(SB, H)
    l_tgt = np.einsum("sh,sh->s", Hm, w_tgt) + b_tgt
    return np.float32((lse - l_tgt).sum() / B)


def kernel(**inputs):
    global _COMPILED
    from concourse.bass_utils import run_bass_kernel_spmd
    in_maps, w_tgt, b_tgt = _prep(inputs)
    if _COMPILED is None:
        _COMPILED = _build()
    res = run_bass_kernel_spmd(_COMPILED, in_maps, list(range(NCORES)))
    return _combine(res.results, w_tgt, b_tgt)


if __name__ == "__main__":
    import reference
    inp = reference.setup_inputs()
    expected = np.asarray(reference.reference(**inp))
    actual = kernel(**{k: np.asarray(v) for k, v in inp.items()})
    err = abs(actual - expected) / max(abs(expected), 1e-9)
    print(f"expected={expected} actual={actual} rel_err={err:.3e}")
